# revision 7
# baseline (speedup 1.0000x reference)
"""DifferenceOfGaussians Trainium2 kernel (8 NeuronCores, SPMD).

Pipeline (per core, channel-sharded with halo recompute):
  pass1: x-conv of the (host-transposed, padded) image as banded Toeplitz
         matmuls on the TensorEngine (float32r), -> rc[y, x]
  pass2: y-conv of rc (banded matmuls, sigma/normalization folded into the
         weights) -> h = sigma*g, streamed per channel
  dog   = h[c] - (s_c/s_{c+1}) h[c+1]    (DVE scalar_tensor_tensor, bf16 out)
  pool  = separable 3x3x3 max over (scale, y, x) in bf16
  lm    = relu(pool + t)   (ScalarE, bf16 out)
  sm    = (dog + (1+t)) - lm  (DVE, fp32 out)

Sharding: core m computes gaussian channels [d0-1, d0+9) (phantom channels at
the ends get zero weights), and outputs pooled dog channels [d0, d0+7).
"""

import sys

sys.path.insert(0, "/opt/trn_rl_repo")

import numpy as np

import concourse.bacc as bacc
import concourse.bass as bass
import concourse.mybir as mybir
from concourse.bass_utils import run_bass_kernel_spmd
from concourse.tile import TileContext

F32 = mybir.dt.float32
F32R = mybir.dt.float32r
BF16 = mybir.dt.bfloat16

H = W = 1024
R = 51          # max radius: int(5 * 10.18 + 0.5)
TAPS = 2 * R + 1
NSIG = 51
PADX = 1152     # padded x rows, 9 tiles of 128 (image occupies [51, 1075))
NT = 8          # y tiles per channel
NCH = 10        # local gaussian channels per core
NDOG = 9        # local dog channels (incl. 1 halo each side)
NOUT = 7        # output dog channels per core (local 1..7)
D0 = [0, 7, 14, 21, 28, 35, 42, 43]
NCORES = 8
LW = 514        # local x width per half
NEG = -1.0e30

_CACHED = {}


def _build_program():
    nc = bacc.Bacc("TRN2", target_bir_lowering=False, debug=False,
                   num_devices=NCORES)

    imgp_d = nc.dram_tensor("imgp", [PADX, W], F32R, kind="ExternalInput")
    b1_d = nc.dram_tensor("b1", [NCH, 3, 128, 258], F32R, kind="ExternalInput")
    b2_d = nc.dram_tensor("b2", [NCH, 2, 128, 128], F32R, kind="ExternalInput")
    ratios_d = nc.dram_tensor("ratios", [128, NDOG], F32, kind="ExternalInput")
    cmask_d = nc.dram_tensor("cmask", [128, NDOG], F32, kind="ExternalInput")
    consts_d = nc.dram_tensor("consts", [128, 2], F32, kind="ExternalInput")
    lm_d = nc.dram_tensor("lm", [NOUT, H, W], BF16, kind="ExternalOutput")
    sm_d = nc.dram_tensor("sm", [NOUT, H, W], F32, kind="ExternalOutput")

    with TileContext(nc) as tc:
        with tc.tile_pool(name="pers", bufs=1) as pers, \
             tc.tile_pool(name="ps1", bufs=2, space="PSUM") as ps1, \
             tc.tile_pool(name="ps2", bufs=2, space="PSUM") as ps2:

            # ---- persistent SBUF ----
            img_t = [pers.tile([128, W], F32R, tag=f"img{t}", name=f"img{t}") for t in range(9)]
            b1_t = [[pers.tile([128, 258], F32R, tag=f"b1_{c}_{p}", name=f"b1_{c}_{p}")
                     for p in range(3)] for c in range(NCH)]
            b2_t = [[pers.tile([128, 128], F32R, tag=f"b2_{c}_{p}", name=f"b2_{c}_{p}")
                     for p in range(2)] for c in range(NCH)]
            ratios = pers.tile([128, NDOG], F32, tag="ratios")
            cmask = pers.tile([128, NDOG], F32, tag="cmask")
            consts = pers.tile([128, 2], F32, tag="consts")

            rc = [pers.tile([128, LW], F32R, tag=f"rc{t}", name=f"rc{t}") for t in range(9)]
            hb = [[pers.tile([128, LW], F32, tag=f"h{r}_{t}", name=f"h{r}_{t}")
                   for t in range(NT)] for r in range(2)]
            dogb = [[pers.tile([128, LW], BF16, tag=f"dog{r}_{t}", name=f"dog{r}_{t}")
                     for t in range(NT)] for r in range(3)]
            cmb = [pers.tile([128, LW], BF16, tag=f"cm{t}", name=f"cm{t}") for t in range(NT)]
            upb = [pers.tile([128, LW], BF16, tag=f"up{t}", name=f"up{t}") for t in range(NT)]
            dnb = [pers.tile([128, LW], BF16, tag=f"dn{t}", name=f"dn{t}") for t in range(NT)]
            ymb = [pers.tile([128, LW], BF16, tag=f"ym{t}", name=f"ym{t}") for t in range(NT)]
            lm_s = [pers.tile([128, LW], BF16, tag=f"lms{r}", name=f"lms{r}") for r in range(2)]
            sm_s = [pers.tile([128, LW], F32, tag=f"sms{r}", name=f"sms{r}") for r in range(2)]

            # ---- load inputs ----
            for t in range(9):
                nc.sync.dma_start(img_t[t][:], imgp_d[t * 128:(t + 1) * 128, :])
            for c in range(NCH):
                for p in range(3):
                    nc.sync.dma_start(b1_t[c][p][:], b1_d[c, p, :, :])
                for p in range(2):
                    nc.sync.dma_start(b2_t[c][p][:], b2_d[c, p, :, :])
            nc.sync.dma_start(ratios[:], ratios_d[:])
            nc.sync.dma_start(cmask[:], cmask_d[:])
            nc.sync.dma_start(consts[:], consts_d[:])

            MX = mybir.AluOpType.max
            ADD = mybir.AluOpType.add
            MUL = mybir.AluOpType.mult
            SUB = mybir.AluOpType.subtract

            def pass1(ci, half):
                """x-conv -> rc tiles (padded y layout)."""
                # zero the y-pad partitions once per channel write
                nc.gpsimd.memset(rc[0][:].bitcast(F32), 0.0)
                nc.gpsimd.memset(rc[8][:].bitcast(F32), 0.0)
                for yb in range(9):
                    if yb == 0:
                        lo, M, ys = 64, 64, (0, 64)
                    elif yb == 8:
                        lo, M, ys = 0, 64, (960, 1024)
                    else:
                        lo, M, ys = 0, 128, (yb * 128 - 64, yb * 128 + 64)
                    psum = ps1.tile([128, 258], F32, tag="p1", name="p1")
                    for bx in range(2):
                        tbase = half * 4 + bx * 2
                        for p in range(3):
                            nc.tensor.matmul(
                                psum[0:M, :],
                                lhsT=img_t[tbase + p][:, ys[0]:ys[1]],
                                rhs=b1_t[ci][p][:],
                                start=(p == 0), stop=(p == 2),
                            )
                        if bx == 0:
                            nc.scalar.copy(rc[yb][lo:lo + M, 0:258], psum[0:M, :])
                        else:
                            nc.scalar.copy(rc[yb][lo:lo + M, 258:514],
                                           psum[0:M, 2:258])

            def pass2(ci):
                """y-conv of rc -> h[ci % 2] (h = sigma * g)."""
                h = hb[ci % 2]
                for yb in range(NT):
                    psum = ps2.tile([128, 1024], F32, tag="p2", name="p2")
                    for xb in range(2):
                        ob = xb * 512  # bank-aligned psum offset
                        xs = xb * 256  # rc column start (blocks overlap by 2)
                        for p in range(2):
                            nc.tensor.matmul(
                                psum[:, ob:ob + 258],
                                lhsT=b2_t[ci][p][:],
                                rhs=rc[yb + p][:, xs:xs + 258],
                                start=(p == 0), stop=(p == 1),
                            )
                    nc.scalar.copy(h[yb][:, 0:258], psum[:, 0:258])
                    nc.scalar.copy(h[yb][:, 258:514], psum[:, 514:770])

            def dog_step(j):
                """dog[j] = h[j] + r_j * h[j+1]  (bf16 out)."""
                hc, hp = hb[(j + 1) % 2], hb[j % 2]
                d = dogb[j % 3]
                for t in range(NT):
                    nc.vector.scalar_tensor_tensor(
                        d[t][:], hc[t][:], ratios[:, j:j + 1], hp[t][:],
                        op0=MUL, op1=ADD)

            def out_step(j, half):
                """channel-max, y-max, x-max, relu, soft-mask, DMA for dog j."""
                dm1, d0_, dp1 = dogb[(j - 1) % 3], dogb[j % 3], dogb[(j + 1) % 3]
                if half == 0:
                    olo, ohi, glo = 0, 513, 0
                else:
                    olo, ohi, glo = 1, 512, 512

                for t in range(NT):
                    nc.vector.scalar_tensor_tensor(
                        cmb[t][:], dm1[t][:], cmask[:, j - 1:j], d0_[t][:],
                        op0=ADD, op1=MX)
                for t in range(NT):
                    nc.vector.scalar_tensor_tensor(
                        cmb[t][:], dp1[t][:], cmask[:, j + 1:j + 2], cmb[t][:],
                        op0=ADD, op1=MX)
                # y-shifted replicas via DMA (compute engines cannot offset partitions)
                for t in range(NT):
                    nc.sync.dma_start(upb[t][0:127, :], cmb[t][1:128, :])
                    if t < NT - 1:
                        nc.sync.dma_start(upb[t][127:128, :], cmb[t + 1][0:1, :])
                    else:
                        nc.sync.dma_start(upb[t][127:128, :], cmb[t][127:128, :])
                    nc.sync.dma_start(dnb[t][1:128, :], cmb[t][0:127, :])
                    if t > 0:
                        nc.sync.dma_start(dnb[t][0:1, :], cmb[t - 1][127:128, :])
                    else:
                        nc.sync.dma_start(dnb[t][0:1, :], cmb[t][0:1, :])
                # ym = max(cm[p-1], cm[p], cm[p+1])
                for t in range(NT):
                    nc.vector.tensor_max(ymb[t][:], cmb[t][:], upb[t][:])
                    nc.vector.tensor_max(ymb[t][:], ymb[t][:], dnb[t][:])
                # x-max (free dim): s2 into upb, xm into dnb
                for t in range(NT):
                    s2 = upb[t]
                    xm = dnb[t]
                    nc.vector.tensor_max(s2[:, 0:513], ymb[t][:, 0:513],
                                         ymb[t][:, 1:514])
                    nc.vector.tensor_max(xm[:, 1:513], s2[:, 0:512],
                                         s2[:, 1:513])
                    if half == 0:
                        nc.vector.tensor_copy(xm[:, 0:1], s2[:, 0:1])
                    else:
                        nc.vector.tensor_copy(xm[:, 511:512], s2[:, 510:511])
                # relu + mask + DMA, per tile (ring buffers)
                for t in range(NT):
                    xm = dnb[t]
                    lm = lm_s[t % 2]
                    sm = sm_s[t % 2]
                    nc.scalar.activation(lm[:, olo:ohi], xm[:, olo:ohi],
                                         mybir.ActivationFunctionType.Relu,
                                         bias=consts[:, 0:1], scale=1.0)
                    nc.vector.scalar_tensor_tensor(
                        sm[:, olo:ohi], d0_[t][:, olo:ohi], consts[:, 1:2],
                        lm[:, olo:ohi], op0=ADD, op1=SUB)
                    rows = slice(t * 128, (t + 1) * 128)
                    cols = slice(glo + olo, glo + ohi)
                    nc.sync.dma_start(lm_d[j - 1, rows, cols], lm[:, olo:ohi])
                    nc.sync.dma_start(sm_d[j - 1, rows, cols], sm[:, olo:ohi])

            for half in range(2):
                for ci in range(NCH):
                    pass1(ci, half)
                    pass2(ci)
                    if ci >= 1:
                        dog_step(ci - 1)
                    if ci >= 3:
                        out_step(ci - 2, half)

    nc.compile()
    return nc


def _tf32(a):
    """Round-to-nearest-even to 10-bit mantissa (tf32 / float32r)."""
    b = np.ascontiguousarray(a, np.float32).view(np.uint32)
    b = (b + np.uint32(0x0FFF) + ((b >> np.uint32(13)) & np.uint32(1))) \
        & np.uint32(0xFFFFE000)
    return b.view(np.float32)


def _host_prep(input, weight, sigmas, threshold):
    x = np.asarray(input, np.float32)[0, 0]          # [H, W]
    w = np.asarray(weight, np.float32)               # [51, 1, 103, 103]
    sig = np.asarray(sigmas, np.float32)             # [51]
    t = float(np.asarray(threshold, np.float32))

    imgp = np.zeros((PADX, W), np.float32)
    imgp[R:R + W, :] = np.ascontiguousarray(x.T)
    imgp = _tf32(imgp)

    wsum = w[:, 0].sum(axis=(1, 2))                  # [51]
    ux = w[:, 0].sum(axis=1)                         # [51, 103] (over ky)
    ay = w[:, 0].sum(axis=2)                         # [51, 103] (over kx)

    n1 = np.arange(258)
    n2 = np.arange(128)
    in_maps = []
    for m in range(NCORES):
        d0 = D0[m]
        b1 = np.zeros((NCH, 3, 128, 258), np.float32)
        b2 = np.zeros((NCH, 2, 128, 128), np.float32)
        ratios = np.zeros(NDOG, np.float32)
        cmaskv = np.zeros(NDOG, np.float32)
        for ci in range(NCH):
            g = d0 - 1 + ci
            if 0 <= g < NSIG:
                T1 = np.zeros((384, 258), np.float32)
                for i in range(TAPS):
                    T1[i + n1, n1] = ux[g, i]
                b1[ci] = _tf32(T1.reshape(3, 128, 258))
                T2 = np.zeros((256, 128), np.float32)
                av = ay[g] * (sig[g] / wsum[g])
                for i in range(TAPS):
                    T2[i + n2 + 13, n2] = av[i]
                b2[ci] = _tf32(T2.reshape(2, 128, 128))
        for j in range(NDOG):
            da = d0 - 1 + j
            if 0 <= da < NSIG - 1:
                ratios[j] = -sig[da] / sig[da + 1]
            else:
                cmaskv[j] = NEG
        in_maps.append({
            "imgp": imgp,
            "b1": b1,
            "b2": b2,
            "ratios": np.tile(ratios, (128, 1)),
            "cmask": np.tile(cmaskv, (128, 1)),
            "consts": np.tile(np.array([t, 1.0 + t], np.float32), (128, 1)),
        })
    return in_maps


def kernel(input, weight, sigmas, threshold, _trace=False, _trace_kwargs=None):
    if "nc" not in _CACHED:
        _CACHED["nc"] = _build_program()
    nc = _CACHED["nc"]
    in_maps = _host_prep(input, weight, sigmas, threshold)
    kw = {}
    if _trace:
        kw = dict(trace=True, **(_trace_kwargs or {}))
    res = run_bass_kernel_spmd(nc, in_maps, list(range(NCORES)), **kw)
    _CACHED["last_result"] = res

    lm = np.empty((NSIG - 1, H, W), np.float32)
    sm = np.empty((NSIG - 1, H, W), np.float32)
    for m in range(NCORES):
        d0 = D0[m]
        lm[d0:d0 + NOUT] = np.asarray(res.results[m]["lm"]).astype(np.float32)
        sm[d0:d0 + NOUT] = np.asarray(res.results[m]["sm"])
    return lm, sm
